# revision 7
# baseline (speedup 1.0000x reference)
"""Trainium2 Bass kernel for nn_Encoder_5248450035714 (2-layer LSTM encoder).

x = emb[input_seq]; two LSTM layers; returns (h_n, c_n) each [2, B, H].
S=256, B=64, E=H=1024, vocab 32000.

Sharding: tensor-parallel over the 4H gate dim across 8 cores. Core c
owns 128 rows of each gate block (order i, g, f, o) => 512 gate cols =>
h-dims [128c, 128c+128). Per step: weight-stationary bf16 matmuls
produce the transposed gate chunk [512, 64] in PSUM; ACT/DVE apply the
LSTM cell; the h-chunk^T [128, 64] is AllGathered so every core has the
full h^T for the next step. Layer-0 input projection is precomputed as
one big GEMM from the gathered x^T; layer-1's input projection is fused
into the per-step matmul (K = 2048 over [ys0_t; h1]).
"""
import os
import sys

sys.path.insert(0, "/opt/trn_rl_repo")

import numpy as np
import ml_dtypes

BF16 = ml_dtypes.bfloat16

S, B, VOCAB, E, H = 256, 64, 32000, 1024, 1024
NCORES = 8
HC = H // NCORES          # 128 h-dims per core
TOK = S * B               # 16384 tokens
KE = E // 128             # 8 contraction chunks over E/H
GATE_ORDER = (0, 2, 1, 3)  # i, g, f, o (block index into the 4H dim)

_CACHE = {}


def _ensure_axon_hooks():
    try:
        import antenv
        if "/opt/trn_rl_repo/antenv" not in list(antenv.__path__):
            antenv.__path__.append("/opt/trn_rl_repo/antenv")
    except Exception:
        pass


def build_nc(n_steps=S):
    import concourse.bacc as bacc
    import concourse.mybir as mybir
    import concourse.tile as tile

    dt = mybir.dt
    AF = mybir.ActivationFunctionType
    nc = bacc.Bacc("TRN2", target_bir_lowering=False, debug=False,
                   num_devices=NCORES)
    ntok = n_steps * B
    NT = min(512, ntok)   # token tile for proj0

    # ---- per-core inputs (host-sharded) ----
    tok = nc.dram_tensor("tok", [128, ntok // 16], dt.int16,
                         kind="ExternalInput")
    embc = nc.dram_tensor("embc", [VOCAB, 128], dt.bfloat16,
                          kind="ExternalInput")
    w_p0 = nc.dram_tensor("w_p0", [KE * 4 * 128, 128], dt.bfloat16,
                          kind="ExternalInput")   # proj0 lhsT tiles [k][m]
    w_r0 = nc.dram_tensor("w_r0", [KE * 4 * 128, 128], dt.bfloat16,
                          kind="ExternalInput")   # rec0 W_hh0^T tiles
    w_r1 = nc.dram_tensor("w_r1", [2 * KE * 4 * 128, 128], dt.bfloat16,
                          kind="ExternalInput")   # rec1 [W_ih1;W_hh1]^T tiles
    b0 = nc.dram_tensor("b0", [4 * 128, 1], dt.float32, kind="ExternalInput")
    b1 = nc.dram_tensor("b1", [4 * 128, 1], dt.float32, kind="ExternalInput")

    out = nc.dram_tensor("out", [4 * 128, B], dt.float32,
                         kind="ExternalOutput")

    xp0 = nc.dram_tensor("xp0", [4 * 128, ntok], dt.float32, kind="Internal")

    rg = [list(range(NCORES))]

    with tile.TileContext(nc) as tc:
        with tc.tile_pool(name="dram", bufs=1, space="DRAM") as dram, \
             tc.tile_pool(name="wpool", bufs=1) as wpool, \
             tc.tile_pool(name="gather", bufs=1) as gpool, \
             tc.tile_pool(name="xtiles", bufs=2) as xpool, \
             tc.tile_pool(name="psum", bufs=1, space="PSUM") as psum_pool, \
             tc.tile_pool(name="ew", bufs=3) as ewpool, \
             tc.tile_pool(name="state", bufs=1) as spool:


            def dma_blocks(dst2d, src, nblk, f):
                """DRAM [(n p), f] -> SBUF [p, (n f)] block-row layout."""
                nc.sync.dma_start(
                    dst2d.rearrange("p (n f) -> p n f", f=f),
                    src.rearrange("(n p) f -> p n f", p=128))

            # ============ Phase 1: gather x^T e-chunk ============
            idx_sb = gpool.tile([128, ntok // 16], dt.int16, tag="idx")
            nc.sync.dma_start(idx_sb[:], tok[:])
            xt_mine = gpool.tile([128, 1, ntok], dt.bfloat16, tag="xt")
            GCH = min(256, ntok)  # idxs per gather instruction
            for gi in range(ntok // GCH):
                nc.gpsimd.dma_gather(
                    xt_mine[:, :, GCH * gi:GCH * (gi + 1)],
                    embc[:],
                    idx_sb[:, (GCH // 16) * gi:(GCH // 16) * (gi + 1)],
                    num_idxs=GCH, num_idxs_reg=GCH, elem_size=128,
                    transpose=True,
                )

            # ============ Phase 2: AllGather x^T ============
            ag_in = dram.tile([128, ntok], dt.bfloat16, tag="agin")
            xt_full = dram.tile([NCORES * 128, ntok], dt.bfloat16, tag="xtf")
            nc.sync.dma_start(ag_in[:], xt_mine[:, 0, :])
            nc.gpsimd.collective_compute(
                "AllGather", mybir.AluOpType.bypass,
                ins=[ag_in.opt()], outs=[xt_full.opt()], replica_groups=rg,
            )

            # ============ Phase 3: proj0 GEMM ============
            w0_sb = wpool.tile([128, KE * 4 * 128], dt.bfloat16, tag="w0")
            dma_blocks(w0_sb[:], w_p0[:], KE * 4, 128)
            b0_sb = wpool.tile([128, 4], dt.float32, tag="b0")
            dma_blocks(b0_sb[:], b0[:], 4, 1)
            for tt in range(ntok // NT):
                rhs = []
                for k in range(KE):
                    r = xpool.tile([128, NT], dt.bfloat16, tag=f"rhs{k}")
                    nc.sync.dma_start(
                        r[:], xt_full[128 * k:128 * (k + 1),
                                      NT * tt:NT * (tt + 1)])
                    rhs.append(r)
                for m in range(4):
                    ps = psum_pool.tile([128, NT], dt.float32, tag=f"ps{m}")
                    for k in range(KE):
                        nc.tensor.matmul(
                            ps[:],
                            w0_sb[:, (k * 4 + m) * 128:(k * 4 + m + 1) * 128],
                            rhs[k][:],
                            start=(k == 0), stop=(k == KE - 1))
                    xo = ewpool.tile([128, NT], dt.float32, tag="xo")
                    nc.scalar.activation(xo[:], ps[:], AF.Identity,
                                         bias=b0_sb[:, m:m + 1])
                    nc.sync.dma_start(
                        xp0[128 * m:128 * (m + 1), NT * tt:NT * (tt + 1)],
                        xo[:])

            # ============ Phases 4+5: recurrences ============
            w0r_sb = wpool.tile([128, KE * 4 * 128], dt.bfloat16, tag="w0r")
            dma_blocks(w0r_sb[:], w_r0[:], KE * 4, 128)
            w1r_sb = wpool.tile([128, 2 * KE * 4 * 128], dt.bfloat16,
                                tag="w1r")
            dma_blocks(w1r_sb[:], w_r1[:], 2 * KE * 4, 128)
            b1_sb = wpool.tile([128, 4], dt.float32, tag="b1")
            dma_blocks(b1_sb[:], b1[:], 4, 1)

            # persistent state
            c0_sb = spool.tile([128, B], dt.float32, tag="c0")
            c1_sb = spool.tile([128, B], dt.float32, tag="c1")
            nc.vector.memset(c0_sb[:], 0.0)
            nc.vector.memset(c1_sb[:], 0.0)
            # double-buffered rhs blocks
            h0_buf = [spool.tile([128, KE * B], dt.bfloat16,
                                 tag=f"h0b{i}", name=f"h0b{i}")
                      for i in range(2)]
            r1_buf = [spool.tile([128, 2 * KE * B], dt.bfloat16,
                                 tag=f"r1b{i}", name=f"r1b{i}")
                      for i in range(2)]

            ys0 = dram.tile([n_steps, NCORES * 128, B], dt.bfloat16,
                            tag="ys0")
            h1ag = dram.tile([2, NCORES * 128, B], dt.bfloat16, tag="h1ag")
            agb0 = dram.tile([2, 128, B], dt.bfloat16, tag="agb0")
            agb1 = dram.tile([2, 128, B], dt.bfloat16, tag="agb1")

            sig, tnh = AF.Sigmoid, AF.Tanh

            def lstm_step(layer, t, rhs_sb, nk_active, c_sb):
                """One LSTM step. rhs_sb: [128, nk*B] bf16 blocks; matmuls
                run over k < nk_active. Returns h_new bf16 [128, B]."""
                w_sb = w0r_sb if layer == 0 else w1r_sb
                ps = psum_pool.tile([128, 4 * B], dt.float32,
                                    tag=f"gps{layer}{t % 2}")
                if layer == 0:
                    xp_sb = ewpool.tile([128, 4 * B], dt.float32,
                                        tag=f"xp{t % 2}")
                    dma_blocks(xp_sb[:], xp0[:, B * t:B * (t + 1)], 4, B)
                for m in range(4):
                    for k in range(nk_active):
                        nc.tensor.matmul(
                            ps[:, B * m:B * (m + 1)],
                            w_sb[:, (k * 4 + m) * 128:(k * 4 + m + 1) * 128],
                            rhs_sb[:, B * k:B * (k + 1)],
                            start=(k == 0), stop=(k == nk_active - 1))
                g_sb = ewpool.tile([128, 4 * B], dt.float32, tag=f"g{layer}")
                for m, fn in ((0, sig), (1, tnh), (2, sig), (3, sig)):
                    sl = slice(B * m, B * (m + 1))
                    if nk_active == 0:
                        # t==0 layer0: gates = xp only (bias folded in)
                        nc.scalar.activation(g_sb[:, sl], xp_sb[:, sl], fn)
                    elif layer == 0:
                        nc.vector.tensor_add(g_sb[:, sl], ps[:, sl],
                                             xp_sb[:, sl])
                        nc.scalar.activation(g_sb[:, sl], g_sb[:, sl], fn)
                    else:
                        nc.scalar.activation(g_sb[:, sl], ps[:, sl], fn,
                                             bias=b1_sb[:, m:m + 1])
                ig = ewpool.tile([128, B], dt.float32, tag=f"ig{layer}")
                nc.vector.tensor_mul(ig[:], g_sb[:, 0:B], g_sb[:, B:2 * B])
                fc = ewpool.tile([128, B], dt.float32, tag=f"fc{layer}")
                nc.vector.tensor_mul(fc[:], g_sb[:, 2 * B:3 * B], c_sb[:])
                nc.vector.tensor_add(c_sb[:], ig[:], fc[:])
                tc_sb = ewpool.tile([128, B], dt.float32, tag=f"tc{layer}")
                nc.scalar.activation(tc_sb[:], c_sb[:], tnh)
                h_new = ewpool.tile([128, B], dt.bfloat16,
                                    tag=f"hn{layer}{t % 2}")
                nc.vector.tensor_mul(h_new[:], g_sb[:, 3 * B:4 * B],
                                     tc_sb[:])
                return h_new

            # ---- rec0 ----
            for t in range(n_steps):
                h0c = lstm_step(0, t, h0_buf[t % 2][:],
                                0 if t == 0 else KE, c0_sb)
                bb = agb0[t % 2, :, :]
                nc.sync.dma_start(bb, h0c[:])
                nc.gpsimd.collective_compute(
                    "AllGather", mybir.AluOpType.bypass,
                    ins=[bb.opt()], outs=[ys0[t, :, :].opt()],
                    replica_groups=rg)
                if t < n_steps - 1:
                    dma_blocks(h0_buf[(t + 1) % 2][:], ys0[t, :, :], KE, B)
                else:
                    h0_final = h0c

            # ---- rec1 ----
            for t in range(n_steps):
                dma_blocks(r1_buf[t % 2][:, 0:KE * B], ys0[t, :, :], KE, B)
                h1c = lstm_step(1, t, r1_buf[t % 2][:],
                                KE if t == 0 else 2 * KE, c1_sb)
                if t < n_steps - 1:
                    bb = agb1[t % 2, :, :]
                    nc.sync.dma_start(bb, h1c[:])
                    nc.gpsimd.collective_compute(
                        "AllGather", mybir.AluOpType.bypass,
                        ins=[bb.opt()], outs=[h1ag[t % 2, :, :].opt()],
                        replica_groups=rg)
                    dma_blocks(r1_buf[(t + 1) % 2][:, KE * B:2 * KE * B],
                               h1ag[t % 2, :, :], KE, B)
                else:
                    h1_final = h1c

            # ---- outputs ----
            of = ewpool.tile([128, B], dt.float32, tag="of")
            nc.scalar.activation(of[:], h0_final[:], AF.Copy)
            nc.sync.dma_start(out[0:128, :], of[:])
            nc.sync.dma_start(out[128:256, :], c0_sb[:])
            of2 = ewpool.tile([128, B], dt.float32, tag="of2")
            nc.scalar.activation(of2[:], h1_final[:], AF.Copy)
            nc.sync.dma_start(out[256:384, :], of2[:])
            nc.sync.dma_start(out[384:512, :], c1_sb[:])

    nc.compile()
    return nc


def _host_prep(inputs, n_steps=S):
    """Build per-core in_maps from full inputs."""
    seq = np.asarray(inputs["input_seq"])[:n_steps].astype(np.int64)
    emb = np.asarray(inputs["emb"], dtype=np.float32)
    ntok = n_steps * B

    toks = seq.reshape(-1).astype(np.int16)  # vocab < 32768
    wrapped = toks.reshape(ntok // 16, 16).T.copy()       # [16, ntok/16]
    wrapped128 = np.tile(wrapped, (8, 1)).astype(np.int16)  # [128, ntok/16]

    w_ih_0T = np.asarray(inputs["w_ih_0"], np.float32).T
    w_hh_0T = np.asarray(inputs["w_hh_0"], np.float32).T
    w1T = np.concatenate([np.asarray(inputs["w_ih_1"], np.float32).T,
                          np.asarray(inputs["w_hh_1"], np.float32).T], axis=0)
    b0sum = (np.asarray(inputs["b_ih_0"], np.float32) +
             np.asarray(inputs["b_hh_0"], np.float32))
    b1sum = (np.asarray(inputs["b_ih_1"], np.float32) +
             np.asarray(inputs["b_hh_1"], np.float32))

    in_maps = []
    for c in range(NCORES):
        m = {"tok": wrapped128,
             "embc": emb[:, 128 * c:128 * (c + 1)].astype(BF16)}

        def tiles(wT, nk):
            cols = np.concatenate(
                [wT[:, H * gb + HC * c: H * gb + HC * (c + 1)]
                 for gb in GATE_ORDER], axis=1)  # [K, 512]
            arr = np.zeros((nk * 4 * 128, 128), dtype=BF16)
            for k in range(nk):
                for mm in range(4):
                    arr[(k * 4 + mm) * 128:(k * 4 + mm + 1) * 128] = \
                        cols[128 * k:128 * (k + 1),
                             128 * mm:128 * (mm + 1)].astype(BF16)
            return arr

        m["w_p0"] = tiles(w_ih_0T, KE)
        m["w_r0"] = tiles(w_hh_0T, KE)
        m["w_r1"] = tiles(w1T, 2 * KE)

        def bias(bsum):
            v = np.concatenate(
                [bsum[H * gb + HC * c: H * gb + HC * (c + 1)]
                 for gb in GATE_ORDER])
            return v.reshape(4 * 128, 1).astype(np.float32)

        m["b0"] = bias(b0sum)
        m["b1"] = bias(b1sum)
        in_maps.append(m)
    return in_maps


def _assemble(results):
    h_n = np.zeros((2, B, H), np.float32)
    c_n = np.zeros((2, B, H), np.float32)
    for c in range(NCORES):
        o = results[c]["out"]
        h_n[0][:, HC * c:HC * (c + 1)] = o[0:128].T
        c_n[0][:, HC * c:HC * (c + 1)] = o[128:256].T
        h_n[1][:, HC * c:HC * (c + 1)] = o[256:384].T
        c_n[1][:, HC * c:HC * (c + 1)] = o[384:512].T
    return h_n, c_n


def run_on_hw(inputs, n_steps=S, trace=False):
    _ensure_axon_hooks()
    from concourse.bass_utils import run_bass_kernel_spmd
    if n_steps not in _CACHE:
        _CACHE[n_steps] = build_nc(n_steps)
    nc = _CACHE[n_steps]
    in_maps = _host_prep(inputs, n_steps)
    res = run_bass_kernel_spmd(nc, in_maps, core_ids=list(range(NCORES)),
                               trace=trace)
    h_n, c_n = _assemble(res.results)
    return (h_n, c_n), res


def kernel(**inputs):
    (h_n, c_n), _ = run_on_hw(inputs, S, trace=False)
    return (h_n, c_n)


if __name__ == "__main__":
    ns = int(os.environ.get("NSTEPS", "4"))
    build_nc(ns)
    print("build OK", ns)


# revision 9
# speedup vs baseline: 1.1135x; 1.1135x over previous
"""Trainium2 Bass kernel for nn_Encoder_5248450035714 (2-layer LSTM encoder).

x = emb[input_seq]; two LSTM layers; returns (h_n, c_n) each [2, B, H].
S=256, B=64, E=H=1024, vocab 32000.

Sharding: tensor-parallel over the 4H gate dim across 8 cores. Core c
owns 128 rows of each gate block (order i, g, f, o) => 512 gate cols =>
h-dims [128c, 128c+128). Per step: weight-stationary bf16 matmuls
produce the transposed gate chunk [512, 64] in PSUM; ACT/DVE apply the
LSTM cell; the h-chunk^T [128, 64] is AllGathered so every core has the
full h^T for the next step. Layer-0 input projection is precomputed as
one big GEMM from the gathered x^T; layer-1's input projection is fused
into the per-step matmul (K = 2048 over [ys0_t; h1]).
"""
import os
import sys

sys.path.insert(0, "/opt/trn_rl_repo")

import numpy as np
import ml_dtypes

BF16 = ml_dtypes.bfloat16

S, B, VOCAB, E, H = 256, 64, 32000, 1024, 1024
NCORES = 8
HC = H // NCORES          # 128 h-dims per core
TOK = S * B               # 16384 tokens
KE = E // 128             # 8 contraction chunks over E/H
GATE_ORDER = (0, 2, 1, 3)  # i, g, f, o (block index into the 4H dim)

_CACHE = {}


def _ensure_axon_hooks():
    try:
        import antenv
        if "/opt/trn_rl_repo/antenv" not in list(antenv.__path__):
            antenv.__path__.append("/opt/trn_rl_repo/antenv")
    except Exception:
        pass


def build_nc(n_steps=S):
    import concourse.bacc as bacc
    import concourse.mybir as mybir
    import concourse.tile as tile

    dt = mybir.dt
    AF = mybir.ActivationFunctionType
    nc = bacc.Bacc("TRN2", target_bir_lowering=False, debug=False,
                   num_devices=NCORES)
    ntok = n_steps * B
    NT = min(512, ntok)   # token tile for proj0

    # ---- per-core inputs (host-sharded) ----
    tok = nc.dram_tensor("tok", [128, ntok // 16], dt.int16,
                         kind="ExternalInput")
    embc = nc.dram_tensor("embc", [VOCAB, 128], dt.bfloat16,
                          kind="ExternalInput")
    w_p0 = nc.dram_tensor("w_p0", [KE * 4 * 128, 128], dt.bfloat16,
                          kind="ExternalInput")   # proj0 lhsT tiles [k][m]
    w_r0 = nc.dram_tensor("w_r0", [KE * 4 * 128, 128], dt.bfloat16,
                          kind="ExternalInput")   # rec0 W_hh0^T tiles
    w_r1 = nc.dram_tensor("w_r1", [2 * KE * 4 * 128, 128], dt.bfloat16,
                          kind="ExternalInput")   # rec1 [W_ih1;W_hh1]^T tiles
    b0 = nc.dram_tensor("b0", [4 * 128, 1], dt.float32, kind="ExternalInput")
    b1 = nc.dram_tensor("b1", [4 * 128, 1], dt.float32, kind="ExternalInput")

    out = nc.dram_tensor("out", [4 * 128, B], dt.float32,
                         kind="ExternalOutput")

    xp0 = nc.dram_tensor("xp0", [4 * 128, ntok], dt.float32, kind="Internal")

    rg = [list(range(NCORES))]

    with tile.TileContext(nc) as tc:
        with tc.tile_pool(name="dram", bufs=1, space="DRAM") as dram, \
             tc.tile_pool(name="wpool", bufs=1) as wpool, \
             tc.tile_pool(name="gather", bufs=1) as gpool, \
             tc.tile_pool(name="xtiles", bufs=2) as xpool, \
             tc.tile_pool(name="psum", bufs=1, space="PSUM") as psum_pool, \
             tc.tile_pool(name="ew", bufs=3) as ewpool, \
             tc.tile_pool(name="state", bufs=1) as spool:


            def dma_blocks(dst2d, src, nblk, f):
                """DRAM [(n p), f] -> SBUF [p, (n f)] block-row layout."""
                nc.sync.dma_start(
                    dst2d.rearrange("p (n f) -> p n f", f=f),
                    src.rearrange("(n p) f -> p n f", p=128))

            # ============ Phase 1: gather x^T e-chunk ============
            idx_sb = gpool.tile([128, ntok // 16], dt.int16, tag="idx")
            nc.sync.dma_start(idx_sb[:], tok[:])
            xt_mine = gpool.tile([128, 1, ntok], dt.bfloat16, tag="xt")
            GCH = min(256, ntok)  # idxs per gather instruction
            for gi in range(ntok // GCH):
                nc.gpsimd.dma_gather(
                    xt_mine[:, :, GCH * gi:GCH * (gi + 1)],
                    embc[:],
                    idx_sb[:, (GCH // 16) * gi:(GCH // 16) * (gi + 1)],
                    num_idxs=GCH, num_idxs_reg=GCH, elem_size=128,
                    transpose=True,
                )

            # ============ Phase 2: AllGather x^T ============
            ag_in = dram.tile([128, ntok], dt.bfloat16, tag="agin")
            xt_full = dram.tile([NCORES * 128, ntok], dt.bfloat16, tag="xtf",
                                addr_space="Shared")
            nc.sync.dma_start(ag_in[:], xt_mine[:, 0, :])
            nc.gpsimd.collective_compute(
                "AllGather", mybir.AluOpType.bypass,
                ins=[ag_in.opt()], outs=[xt_full.opt()], replica_groups=rg,
            )

            # ============ Phase 3: proj0 GEMM ============
            w0_sb = wpool.tile([128, KE * 4 * 128], dt.bfloat16, tag="w0")
            dma_blocks(w0_sb[:], w_p0[:], KE * 4, 128)
            b0_sb = wpool.tile([128, 4], dt.float32, tag="b0")
            dma_blocks(b0_sb[:], b0[:], 4, 1)
            for tt in range(ntok // NT):
                rhs = []
                for k in range(KE):
                    r = xpool.tile([128, NT], dt.bfloat16, tag=f"rhs{k}")
                    nc.sync.dma_start(
                        r[:], xt_full[128 * k:128 * (k + 1),
                                      NT * tt:NT * (tt + 1)])
                    rhs.append(r)
                for m in range(4):
                    ps = psum_pool.tile([128, NT], dt.float32, tag=f"ps{m}")
                    for k in range(KE):
                        nc.tensor.matmul(
                            ps[:],
                            w0_sb[:, (k * 4 + m) * 128:(k * 4 + m + 1) * 128],
                            rhs[k][:],
                            start=(k == 0), stop=(k == KE - 1))
                    xo = ewpool.tile([128, NT], dt.float32, tag="xo")
                    nc.scalar.activation(xo[:], ps[:], AF.Identity,
                                         bias=b0_sb[:, m:m + 1])
                    nc.sync.dma_start(
                        xp0[128 * m:128 * (m + 1), NT * tt:NT * (tt + 1)],
                        xo[:])

            # ============ Phases 4+5: recurrences ============
            w0r_sb = wpool.tile([128, KE * 4 * 128], dt.bfloat16, tag="w0r")
            dma_blocks(w0r_sb[:], w_r0[:], KE * 4, 128)
            w1r_sb = wpool.tile([128, 2 * KE * 4 * 128], dt.bfloat16,
                                tag="w1r")
            dma_blocks(w1r_sb[:], w_r1[:], 2 * KE * 4, 128)
            b1_sb = wpool.tile([128, 4], dt.float32, tag="b1")
            dma_blocks(b1_sb[:], b1[:], 4, 1)

            # persistent state
            c0_sb = spool.tile([128, B], dt.float32, tag="c0")
            c1_sb = spool.tile([128, B], dt.float32, tag="c1")
            nc.vector.memset(c0_sb[:], 0.0)
            nc.vector.memset(c1_sb[:], 0.0)
            # double-buffered rhs blocks
            h0_buf = [spool.tile([128, KE * B], dt.bfloat16,
                                 tag=f"h0b{i}", name=f"h0b{i}")
                      for i in range(2)]
            r1_buf = [spool.tile([128, 2 * KE * B], dt.bfloat16,
                                 tag=f"r1b{i}", name=f"r1b{i}")
                      for i in range(2)]

            ys0 = [dram.tile([NCORES * 128, B], dt.bfloat16,
                              tag=f"ys0_{t}", name=f"ys0_{t}",
                              addr_space="Shared")
                   for t in range(n_steps)]
            h1ag = [dram.tile([NCORES * 128, B], dt.bfloat16,
                              tag=f"h1ag_{t}", name=f"h1ag_{t}",
                              addr_space="Shared")
                    for t in range(max(n_steps - 1, 1))]
            agb0 = dram.tile([2, 128, B], dt.bfloat16, tag="agb0")
            agb1 = dram.tile([2, 128, B], dt.bfloat16, tag="agb1")

            sig, tnh = AF.Sigmoid, AF.Tanh

            def lstm_step(layer, t, rhs_sb, nk_active, c_sb):
                """One LSTM step. rhs_sb: [128, nk*B] bf16 blocks; matmuls
                run over k < nk_active. Returns h_new bf16 [128, B]."""
                w_sb = w0r_sb if layer == 0 else w1r_sb
                ps = psum_pool.tile([128, 4 * B], dt.float32,
                                    tag=f"gps{layer}{t % 2}")
                if layer == 0:
                    xp_sb = ewpool.tile([128, 4 * B], dt.float32,
                                        tag=f"xp{t % 2}")
                    dma_blocks(xp_sb[:], xp0[:, B * t:B * (t + 1)], 4, B)
                for m in range(4):
                    for k in range(nk_active):
                        nc.tensor.matmul(
                            ps[:, B * m:B * (m + 1)],
                            w_sb[:, (k * 4 + m) * 128:(k * 4 + m + 1) * 128],
                            rhs_sb[:, B * k:B * (k + 1)],
                            start=(k == 0), stop=(k == nk_active - 1))
                g_sb = ewpool.tile([128, 4 * B], dt.float32, tag=f"g{layer}")
                for m, fn in ((0, sig), (1, tnh), (2, sig), (3, sig)):
                    sl = slice(B * m, B * (m + 1))
                    if nk_active == 0:
                        # t==0 layer0: gates = xp only (bias folded in)
                        nc.scalar.activation(g_sb[:, sl], xp_sb[:, sl], fn)
                    elif layer == 0:
                        nc.vector.tensor_add(g_sb[:, sl], ps[:, sl],
                                             xp_sb[:, sl])
                        nc.scalar.activation(g_sb[:, sl], g_sb[:, sl], fn)
                    else:
                        nc.scalar.activation(g_sb[:, sl], ps[:, sl], fn,
                                             bias=b1_sb[:, m:m + 1])
                ig = ewpool.tile([128, B], dt.float32, tag=f"ig{layer}")
                nc.vector.tensor_mul(ig[:], g_sb[:, 0:B], g_sb[:, B:2 * B])
                fc = ewpool.tile([128, B], dt.float32, tag=f"fc{layer}")
                nc.vector.tensor_mul(fc[:], g_sb[:, 2 * B:3 * B], c_sb[:])
                nc.vector.tensor_add(c_sb[:], ig[:], fc[:])
                tc_sb = ewpool.tile([128, B], dt.float32, tag=f"tc{layer}")
                nc.scalar.activation(tc_sb[:], c_sb[:], tnh)
                h_new = ewpool.tile([128, B], dt.bfloat16,
                                    tag=f"hn{layer}{t % 2}")
                nc.vector.tensor_mul(h_new[:], g_sb[:, 3 * B:4 * B],
                                     tc_sb[:])
                return h_new

            # ---- interleaved rec0 (step tau) + rec1 (step tau-1) ----
            # AG0(tau) overlaps rec1 step tau-1; AG1(tau-1) overlaps
            # rec0 step tau+1.
            for tau in range(n_steps + 1):
                if tau < n_steps:
                    h0c = lstm_step(0, tau, h0_buf[tau % 2][:],
                                    0 if tau == 0 else KE, c0_sb)
                    bb = agb0[tau % 2, :, :]
                    nc.sync.dma_start(bb, h0c[:])
                    nc.gpsimd.collective_compute(
                        "AllGather", mybir.AluOpType.bypass,
                        ins=[bb.opt()], outs=[ys0[tau].opt()],
                        replica_groups=rg)
                    if tau < n_steps - 1:
                        dma_blocks(h0_buf[(tau + 1) % 2][:], ys0[tau][:],
                                   KE, B)
                    else:
                        h0_final = h0c
                    # x-part for rec1 step tau (consumed next tick)
                    dma_blocks(r1_buf[tau % 2][:, 0:KE * B], ys0[tau][:],
                               KE, B)
                if tau >= 1:
                    t1 = tau - 1
                    h1c = lstm_step(1, t1, r1_buf[t1 % 2][:],
                                    KE if t1 == 0 else 2 * KE, c1_sb)
                    if t1 < n_steps - 1:
                        bb = agb1[t1 % 2, :, :]
                        nc.sync.dma_start(bb, h1c[:])
                        nc.gpsimd.collective_compute(
                            "AllGather", mybir.AluOpType.bypass,
                            ins=[bb.opt()],
                            outs=[h1ag[t1].opt()],
                            replica_groups=rg)
                        dma_blocks(
                            r1_buf[(t1 + 1) % 2][:, KE * B:2 * KE * B],
                            h1ag[t1][:], KE, B)
                    else:
                        h1_final = h1c

            # ---- outputs ----
            of = ewpool.tile([128, B], dt.float32, tag="of")
            nc.scalar.activation(of[:], h0_final[:], AF.Copy)
            nc.sync.dma_start(out[0:128, :], of[:])
            nc.sync.dma_start(out[128:256, :], c0_sb[:])
            of2 = ewpool.tile([128, B], dt.float32, tag="of2")
            nc.scalar.activation(of2[:], h1_final[:], AF.Copy)
            nc.sync.dma_start(out[256:384, :], of2[:])
            nc.sync.dma_start(out[384:512, :], c1_sb[:])

    nc.compile()
    return nc


def _host_prep(inputs, n_steps=S):
    """Build per-core in_maps from full inputs."""
    seq = np.asarray(inputs["input_seq"])[:n_steps].astype(np.int64)
    emb = np.asarray(inputs["emb"], dtype=np.float32)
    ntok = n_steps * B

    toks = seq.reshape(-1).astype(np.int16)  # vocab < 32768
    wrapped = toks.reshape(ntok // 16, 16).T.copy()       # [16, ntok/16]
    wrapped128 = np.tile(wrapped, (8, 1)).astype(np.int16)  # [128, ntok/16]

    w_ih_0T = np.asarray(inputs["w_ih_0"], np.float32).T
    w_hh_0T = np.asarray(inputs["w_hh_0"], np.float32).T
    w1T = np.concatenate([np.asarray(inputs["w_ih_1"], np.float32).T,
                          np.asarray(inputs["w_hh_1"], np.float32).T], axis=0)
    b0sum = (np.asarray(inputs["b_ih_0"], np.float32) +
             np.asarray(inputs["b_hh_0"], np.float32))
    b1sum = (np.asarray(inputs["b_ih_1"], np.float32) +
             np.asarray(inputs["b_hh_1"], np.float32))

    in_maps = []
    for c in range(NCORES):
        m = {"tok": wrapped128,
             "embc": emb[:, 128 * c:128 * (c + 1)].astype(BF16)}

        def tiles(wT, nk):
            cols = np.concatenate(
                [wT[:, H * gb + HC * c: H * gb + HC * (c + 1)]
                 for gb in GATE_ORDER], axis=1)  # [K, 512]
            arr = np.zeros((nk * 4 * 128, 128), dtype=BF16)
            for k in range(nk):
                for mm in range(4):
                    arr[(k * 4 + mm) * 128:(k * 4 + mm + 1) * 128] = \
                        cols[128 * k:128 * (k + 1),
                             128 * mm:128 * (mm + 1)].astype(BF16)
            return arr

        m["w_p0"] = tiles(w_ih_0T, KE)
        m["w_r0"] = tiles(w_hh_0T, KE)
        m["w_r1"] = tiles(w1T, 2 * KE)

        def bias(bsum):
            v = np.concatenate(
                [bsum[H * gb + HC * c: H * gb + HC * (c + 1)]
                 for gb in GATE_ORDER])
            return v.reshape(4 * 128, 1).astype(np.float32)

        m["b0"] = bias(b0sum)
        m["b1"] = bias(b1sum)
        in_maps.append(m)
    return in_maps


def _assemble(results):
    h_n = np.zeros((2, B, H), np.float32)
    c_n = np.zeros((2, B, H), np.float32)
    for c in range(NCORES):
        o = results[c]["out"]
        h_n[0][:, HC * c:HC * (c + 1)] = o[0:128].T
        c_n[0][:, HC * c:HC * (c + 1)] = o[128:256].T
        h_n[1][:, HC * c:HC * (c + 1)] = o[256:384].T
        c_n[1][:, HC * c:HC * (c + 1)] = o[384:512].T
    return h_n, c_n


def run_on_hw(inputs, n_steps=S, trace=False):
    _ensure_axon_hooks()
    from concourse.bass_utils import run_bass_kernel_spmd
    if n_steps not in _CACHE:
        _CACHE[n_steps] = build_nc(n_steps)
    nc = _CACHE[n_steps]
    in_maps = _host_prep(inputs, n_steps)
    res = run_bass_kernel_spmd(nc, in_maps, core_ids=list(range(NCORES)),
                               trace=trace)
    h_n, c_n = _assemble(res.results)
    return (h_n, c_n), res


def kernel(**inputs):
    (h_n, c_n), _ = run_on_hw(inputs, S, trace=False)
    return (h_n, c_n)


if __name__ == "__main__":
    ns = int(os.environ.get("NSTEPS", "4"))
    build_nc(ns)
    print("build OK", ns)
